# revision 1
# baseline (speedup 1.0000x reference)
"""KDE log-density kernel for Trainium2, SPMD across 8 NeuronCores.

Computes log_p[m] = logsumexp_n(-scale * ||X[m] - svs[n]||^2) - log(N)
                    + (D/2) * log(scale/pi)

Sharding: X rows split across 8 cores (1024 rows each); svs + scale
replicated. logsumexp over N is local to each row, no collectives.

Per-core algorithm:
  - Pass over svs: s2[n] = sum_d svs[n,d]^2 (ACT Square+accum), cast
    2*svs to bf16 and bounce through DRAM for a DMA transpose into
    svsT [128(d), 4, 8192(n)] resident in SBUF.
  - Same for X: x2, bf16 X^T tiles.
  - s2 broadcast to all 128 partitions via identity-transpose + rank-1
    matmuls -> s2b [128, 8192] f32.
  - Main loop over 8 m-tiles x 16 n-chunks: 4 matmuls (K=128 each)
    accumulate psum = 2*X.svs; DVE tensor_tensor_reduce computes
    u = psum - s2[n] in-place and max_n(u) in one op; online-max
    rescale (flash style); ACT computes exp(scale*u + bias) in-place
    with a fused row-sum accum_out.
  - log_p = ln(acc) + M + C,  C = -ln(N) + (D/2)*(ln(scale) - ln(pi)).
"""

import sys

for _p in ("/opt/trn_rl_repo", "/opt/pypackages"):
    if _p not in sys.path:
        sys.path.insert(0, _p)

import numpy as np

M_FULL, N, D = 8192, 8192, 512
NCORES = 8
M_LOC = M_FULL // NCORES  # 1024
P = 128
KS = D // P               # 4 k-subtiles
MT = M_LOC // P           # 8 m-tiles per core
NCH = 512                 # n-chunk (one PSUM bank of f32)
NCHUNKS = N // NCH        # 16

_CACHE = {}
USE_ONLINE_MAX = False


def _build_nc():
    import concourse.mybir as mybir
    import concourse.tile as tile
    from concourse import bacc

    f32 = mybir.dt.float32
    bf16 = mybir.dt.bfloat16
    AF = mybir.ActivationFunctionType
    ALU = mybir.AluOpType
    AX = mybir.AxisListType

    nc = bacc.Bacc(None, target_bir_lowering=False, debug=True)
    Xd = nc.declare_dram_parameter("X", [M_LOC, D], f32, isOutput=False)
    Sd = nc.declare_dram_parameter("svs", [N, D], f32, isOutput=False)
    scd = nc.declare_dram_parameter("scale", [1], f32, isOutput=False)
    outd = nc.declare_dram_parameter("out", [M_LOC, 1], f32, isOutput=True)

    LOG_CONST = float(-np.log(N) - (D / 2) * np.log(np.pi))

    with tile.TileContext(nc) as tc:
        with (
            tc.tile_pool(name="const", bufs=1) as cp,
            tc.tile_pool(name="work", bufs=3) as wp,
            tc.tile_pool(name="small", bufs=4) as sp,
            tc.tile_pool(name="mmpsum", bufs=6, space="PSUM") as pp,
            tc.tile_pool(name="bcpsum", bufs=2, space="PSUM") as pb,
            tc.tile_pool(name="dram", bufs=1, space="DRAM") as dp,
        ):
            # ---------------- constants ----------------
            scale_bc = cp.tile([P, 1], f32)
            nc.sync.dma_start(scale_bc[:], scd[None, :].to_broadcast((P, 1)))
            negscale = cp.tile([P, 1], f32)
            nc.scalar.mul(negscale[:], scale_bc[:], -1.0)
            # C = (D/2)*ln(scale) + (-ln(N) - (D/2)*ln(pi))
            C_bc = cp.tile([P, 1], f32)
            nc.scalar.activation(C_bc[:], scale_bc[:], AF.Ln)
            nc.vector.tensor_scalar(
                C_bc[:], C_bc[:], float(D / 2), LOG_CONST, ALU.mult, ALU.add
            )

            # ---------------- svs pass: s2 + bf16(2*svs) ----------------
            # chunk c holds svs rows {p*64 + c}: s2_all[p, c] = s2(p*64+c)
            # flattens p-major directly into natural n order.
            NC64 = N // P  # 64 chunks
            s2_all = cp.tile([P, NC64], f32)
            svs2b = dp.tile([N, D], bf16)
            for c in range(NC64):
                sv = wp.tile([P, D], f32, tag="in_f32")
                nc.sync.dma_start(sv[:], Sd[c::NC64, :])
                sq = wp.tile([P, D], f32, tag="sq")
                nc.scalar.activation(
                    sq[:], sv[:], AF.Square, accum_out=s2_all[:, c:c + 1]
                )
                svb = wp.tile([P, D], bf16, tag="out_b16")
                nc.vector.tensor_scalar_mul(svb[:], sv[:], 2.0)
                nc.sync.dma_start(svs2b[c::NC64, :], svb[:])

            s2_dram = dp.tile([N], f32)
            nc.sync.dma_start(s2_dram.rearrange("(p c) -> p c", p=P), s2_all[:])
            # broadcast s2 to all partitions via stride-0 DMA
            s2b = cp.tile([P, N], f32)
            nc.sync.dma_start(s2b[:], s2_dram[None, :].to_broadcast((P, N)))

            # ---------------- X pass: x2 + bf16(X) ----------------
            x2sc = cp.tile([P, MT], f32)  # -scale * x2, per m-tile column
            x2_all = cp.tile([P, MT], f32)
            xb_dram = dp.tile([M_LOC, D], bf16)
            for t in range(MT):
                xv = wp.tile([P, D], f32, tag="in_f32")
                nc.sync.dma_start(xv[:], Xd[t * P:(t + 1) * P, :])
                xsq = wp.tile([P, D], f32, tag="sq")
                nc.scalar.activation(
                    xsq[:], xv[:], AF.Square, accum_out=x2_all[:, t:t + 1]
                )
                xbv = wp.tile([P, D], bf16, tag="out_b16")
                nc.vector.tensor_copy(xbv[:], xv[:])
                nc.sync.dma_start(xb_dram[t * P:(t + 1) * P, :], xbv[:])
                nc.vector.tensor_tensor(
                    x2sc[:, t:t + 1], x2_all[:, t:t + 1], negscale[:], ALU.mult
                )

            # ---------------- DMA transposes (bf16) ----------------
            svsT = cp.tile([P, KS, N], bf16)   # [d%128, d//128, n]
            nc.sync.dma_start_transpose(svsT[:], svs2b[:])
            xT = cp.tile([P, KS, M_LOC], bf16)
            nc.sync.dma_start_transpose(xT[:], xb_dram[:])

            # ---------------- main loop ----------------
            if USE_ONLINE_MAX:
                for t in range(MT):
                    B_t = sp.tile([P, 1], f32, tag=f"B{t}")
                    acc_t = sp.tile([P, 1], f32, tag=f"acc{t}")
                    for j in range(NCHUNKS):
                        ps = pp.tile([P, NCH], f32, tag="mm")
                        for kt in range(KS):
                            nc.tensor.matmul(
                                ps[:],
                                xT[:, kt, t * P:(t + 1) * P],
                                svsT[:, kt, j * NCH:(j + 1) * NCH],
                                start=(kt == 0),
                                stop=(kt == KS - 1),
                            )
                        mxu = sp.tile([P, 1], f32, tag="mxu")
                        uu = wp.tile([P, NCH], f32, tag="uu")
                        nc.vector.tensor_tensor_reduce(
                            out=uu[:],
                            in0=ps[:],
                            in1=s2b[:, j * NCH:(j + 1) * NCH],
                            scale=1.0,
                            scalar=-3.4e38,
                            op0=ALU.subtract,
                            op1=ALU.max,
                            accum_out=mxu[:],
                        )
                        nm = sp.tile([P, 1], f32, tag="nm")
                        nc.vector.tensor_tensor(nm[:], mxu[:], negscale[:], ALU.mult)
                        pj = sp.tile([P, 1], f32, tag="pj")
                        ee = wp.tile([P, NCH], f32, tag="ee")
                        if j == 0:
                            nc.vector.tensor_copy(B_t[:], nm[:])
                            nc.scalar.activation(
                                ee[:], uu[:], AF.Exp,
                                bias=B_t[:], scale=scale_bc[:], accum_out=pj[:],
                            )
                            nc.vector.tensor_copy(acc_t[:], pj[:])
                        else:
                            dd = sp.tile([P, 1], f32, tag="dd")
                            nc.vector.tensor_scalar(
                                dd[:], nm[:], B_t[:], 0.0, ALU.subtract, ALU.min
                            )
                            nc.vector.tensor_tensor(B_t[:], B_t[:], dd[:], ALU.add)
                            rr = sp.tile([P, 1], f32, tag="rr")
                            nc.scalar.activation(rr[:], dd[:], AF.Exp)
                            nc.scalar.activation(
                                ee[:], uu[:], AF.Exp,
                                bias=B_t[:], scale=scale_bc[:], accum_out=pj[:],
                            )
                            nc.vector.tensor_scalar(
                                acc_t[:], acc_t[:], rr[:], pj[:], ALU.mult, ALU.add
                            )
                    mfin = sp.tile([P, 1], f32, tag="mfin")
                    nc.vector.tensor_tensor(
                        mfin[:], x2sc[:, t:t + 1], B_t[:], ALU.subtract
                    )
                    nc.vector.tensor_tensor(mfin[:], mfin[:], C_bc[:], ALU.add)
                    lp = sp.tile([P, 1], f32, tag="lp")
                    nc.scalar.activation(lp[:], acc_t[:], AF.Ln)
                    nc.vector.tensor_tensor(lp[:], lp[:], mfin[:], ALU.add)
                    nc.sync.dma_start(outd[t * P:(t + 1) * P, :], lp[:])
            else:
                for t in range(MT):
                    partials = sp.tile([P, NCHUNKS], f32, tag=f"part{t}")
                    for j in range(NCHUNKS):
                        ps = pp.tile([P, NCH], f32, tag="mm")
                        for kt in range(KS):
                            nc.tensor.matmul(
                                ps[:],
                                xT[:, kt, t * P:(t + 1) * P],
                                svsT[:, kt, j * NCH:(j + 1) * NCH],
                                start=(kt == 0),
                                stop=(kt == KS - 1),
                            )
                        uu = wp.tile([P, NCH], f32, tag="uu")
                        nc.vector.tensor_tensor(
                            uu[:], ps[:], s2b[:, j * NCH:(j + 1) * NCH],
                            ALU.subtract,
                        )
                        ee = wp.tile([P, NCH], f32, tag="ee")
                        nc.scalar.activation(
                            ee[:], uu[:], AF.Exp,
                            bias=x2sc[:, t:t + 1], scale=scale_bc[:],
                            accum_out=partials[:, j:j + 1],
                        )
                    S_t = sp.tile([P, 1], f32, tag="S")
                    nc.vector.reduce_sum(S_t[:], partials[:], axis=AX.X)
                    lp = sp.tile([P, 1], f32, tag="lp")
                    nc.scalar.activation(lp[:], S_t[:], AF.Ln)
                    nc.vector.tensor_tensor(lp[:], lp[:], C_bc[:], ALU.add)
                    nc.sync.dma_start(outd[t * P:(t + 1) * P, :], lp[:])

    nc.finalize()
    return nc


def kernel(X: np.ndarray, svs: np.ndarray, scale: np.ndarray) -> np.ndarray:
    from concourse.bass_utils import run_bass_kernel_spmd

    if "nc" not in _CACHE:
        _CACHE["nc"] = _build_nc()
    nc = _CACHE["nc"]

    X = np.ascontiguousarray(X, dtype=np.float32)
    svs = np.ascontiguousarray(svs, dtype=np.float32)
    sc = np.asarray(scale, dtype=np.float32).reshape(1)

    in_maps = [
        {"X": X[i * M_LOC:(i + 1) * M_LOC], "svs": svs, "scale": sc}
        for i in range(NCORES)
    ]
    res = run_bass_kernel_spmd(nc, in_maps, core_ids=list(range(NCORES)))
    out = np.concatenate([r["out"].reshape(M_LOC) for r in res.results])
    return out.astype(np.float32)



# revision 16
# speedup vs baseline: 2.1344x; 2.1344x over previous
"""KDE log-density kernel for Trainium2, SPMD across 8 NeuronCores.

Computes log_p[m] = logsumexp_n(-scale * ||X[m] - svs[n]||^2) - log(N)
                    + (D/2) * log(scale/pi)

Sharding: 4-way over X rows x 2-way over svs rows (core i handles X
quarter i%4 against svs half i//4).  Each core returns the raw partial
sum T[m] = sum_n exp(-scale*||x_m - s_n||^2) over its svs half; the
host unshards by summing the two halves per query row and applying
log(T) + C.  This halves per-core DMA versus replicating svs, which is
what the kernel is otherwise bound by.

Per-core algorithm (fp8 DoubleRow pipeline):
  - svs (2 chunks of 2048 rows = one n-group each): DMA f32 load ->
    DVE cast 2*svs to fp8e4 -> DMA store fp8 -> DMA-transpose the
    uint16-bitcast (adjacent-d pairs packed per 16-bit element) into
    per-group svsT8 [128, 2, 2*NG] fp8 where (partition p, K, lane i)
    maps to d = 2*(K*128+p)+i.  Squares of svsT8 (for the -s2 inject)
    alternate Pool/DVE per 512-n slice.  Identical packing on both
    matmul operands keeps the DoubleRow contraction consistent.
  - X: f32 load in halves; x2 row-sums for half 0 on the idle ACT head
    (Square+accum), half 1 on DVE; cast/store/transpose per half.
  - Main loop over 2 n-groups x 16 m-tiles: per 512-n chunk, 2
    DoubleRow matmuls (K=256 each) accumulate 2*x.s into a psum bank,
    then 2 DoubleRow matmuls with an all-(-0.25) stationary against
    sqT8 add -s2[n] into the same bank.  One wide ACT exp over 4 banks
    [128, 2048] in-place with bias=-scale*x2[m], scale=scale,
    accum_out -> partials.  T = sum of the 2 group partials.

DMA queues: loads with no dependencies on SP; dependent stores and
transposes issue from the ACT queue (issue-only on its sequencer).
"""

import sys

for _p in ("/opt/trn_rl_repo", "/opt/pypackages"):
    if _p not in sys.path:
        sys.path.insert(0, _p)

import numpy as np

M_FULL, N_FULL, D = 8192, 8192, 512
NCORES = 8
XSH, SSH = 4, 2           # X shards x svs shards
M_LOC = M_FULL // XSH     # 2048
N_LOC = N_FULL // SSH     # 4096
P = 128
MT = M_LOC // P           # 16 m-tiles per core
NCH = 512                 # psum bank of f32
NGRP = 2                  # n-groups (one wide psum tile each per m-tile)
NG = N_LOC // NGRP        # 2048 n per group
RCH = 2048                # svs rows per pipeline chunk (= one n-group)
JCH = RCH // P            # 16 row-tiles per chunk
XH = M_LOC // 2           # X half rows (1024)

_CACHE = {}


def _build_nc():
    import concourse.mybir as mybir
    import concourse.tile as tile
    from concourse import bacc

    f32 = mybir.dt.float32
    fp8 = mybir.dt.float8e4
    u16 = mybir.dt.uint16
    AF = mybir.ActivationFunctionType
    ALU = mybir.AluOpType

    DR = mybir.MatmulPerfMode.DoubleRow

    nc = bacc.Bacc(None, target_bir_lowering=False, debug=True)
    Xd = nc.declare_dram_parameter("X", [M_LOC, D], f32, isOutput=False)
    Sd = nc.declare_dram_parameter("svs", [N_LOC, D], f32, isOutput=False)
    scd = nc.declare_dram_parameter("scale", [1], f32, isOutput=False)
    outd = nc.declare_dram_parameter("out", [M_LOC], f32, isOutput=True)

    def drpair(ap2d, n0, ncols):
        # fp8 [128, 2*cols] packed-pair slice -> DoubleRow [128, 2, ncols]
        return ap2d[:, 2 * n0:2 * (n0 + ncols)].rearrange(
            "p (n two) -> p two n", two=2
        )

    with tile.TileContext(nc) as tc:
        with (
            tc.tile_pool(name="const", bufs=1) as cp,
            tc.tile_pool(name="stage", bufs=2) as stp,
            tc.tile_pool(name="s8", bufs=2) as s8p,
            tc.tile_pool(name="small", bufs=4) as sp,
            tc.tile_pool(name="wpsum", bufs=2, space="PSUM") as pp,
            tc.tile_pool(name="dram", bufs=1, space="DRAM") as dp,
        ):
            # ---------- constants ----------
            scale_bc = cp.tile([P, 1], f32)
            nc.sync.dma_start(scale_bc[:], scd[None, :].to_broadcast((P, 1)))
            negscale = cp.tile([P, 1], f32)
            nc.scalar.mul(negscale[:], scale_bc[:], -1.0)
            neg_q = cp.tile([P, 2, P], fp8)
            nc.gpsimd.memset(neg_q[:], -0.25)

            # resident tensors
            xst = cp.tile([P, MT, D], f32)          # X f32, row t*128+p
            x8 = cp.tile([P, MT, D], fp8)
            xT8 = cp.tile([P, 2, M_LOC * 2], fp8)   # packed pairs
            xT8p = cp.tile([P, 2, 2, M_LOC], fp8)   # planar [p, K, lane, m]
            x2_all = cp.tile([P, MT], f32)
            x2sc = cp.tile([P, MT], f32)
            svsT8 = [cp.tile([P, 2, NG * 2], fp8, name=f"svsT8{g}")
                     for g in range(NGRP)]
            sqT8 = [cp.tile([P, 2, NG * 2], fp8, name=f"sqT8{g}")
                    for g in range(NGRP)]
            partials = cp.tile([P, MT, NGRP], f32)

            x8d = dp.tile([M_LOC, D], fp8)
            s8d = dp.tile([N_LOC, D], fp8)

            # ---------- pipeline stages ----------
            def x_load(h):
                nc.sync.dma_start(
                    xst[:, h * 8:(h + 1) * 8, :],
                    Xd[h * XH:(h + 1) * XH, :].rearrange(
                        "(t p) d -> p t d", p=P),
                )

            def x_cast(h):
                nc.vector.tensor_copy(
                    x8[:, h * 8:(h + 1) * 8, :], xst[:, h * 8:(h + 1) * 8, :]
                )

            def x_store_transp(h):
                nc.scalar.dma_start(
                    x8d[h * XH:(h + 1) * XH, :].rearrange(
                        "(t p) d -> p t d", p=P),
                    x8[:, h * 8:(h + 1) * 8, :],
                )
                nc.scalar.dma_start_transpose(
                    xT8.bitcast(u16)[:, :, h * XH:(h + 1) * XH],
                    x8d.bitcast(u16)[h * XH:(h + 1) * XH, :],
                )
                # de-interleave to planar for the dual-fp8 ldweights ISA
                for K in range(2):
                    for i in range(2):
                        nc.vector.tensor_copy(
                            xT8p[:, K, i, h * XH:(h + 1) * XH],
                            xT8[:, K, 2 * h * XH + i:2 * (h + 1) * XH:2],
                        )

            def x_sq_act(h):
                for t in range(h * 8, (h + 1) * 8):
                    xsq = sp.tile([P, D], f32, tag="xsq")
                    nc.scalar.activation(
                        xsq[:], xst[:, t, :], AF.Square,
                        accum_out=x2_all[:, t:t + 1],
                    )

            def x_sq_dve(h):
                for t in range(h * 8, (h + 1) * 8):
                    xsq = sp.tile([P, D], fp8, tag="xsq8")
                    nc.vector.scalar_tensor_tensor(
                        xsq[:], xst[:, t, :], 1.0, xst[:, t, :],
                        ALU.mult, ALU.mult, accum_out=x2_all[:, t:t + 1],
                    )

            def x2sc_piece(h, eng):
                eng.tensor_scalar(
                    x2sc[:, h * 8:(h + 1) * 8],
                    x2_all[:, h * 8:(h + 1) * 8],
                    negscale[:], 0.0, ALU.mult, ALU.add,
                )

            sv_stage = {}

            def sv_load(ch):
                svst = stp.tile([P, JCH, D], f32, tag="svst")
                sv_stage[ch] = svst
                nc.sync.dma_start(
                    svst[:],
                    Sd[ch * RCH:(ch + 1) * RCH, :].rearrange(
                        "(j p) d -> p j d", p=P),
                )

            def sv_cast_store(ch):
                sv8 = s8p.tile([P, JCH, D], fp8, tag="sv8")
                nc.vector.tensor_scalar_mul(sv8[:], sv_stage.pop(ch)[:], 2.0)
                nc.scalar.dma_start(
                    s8d[ch * RCH:(ch + 1) * RCH, :].rearrange(
                        "(j p) d -> p j d", p=P),
                    sv8[:],
                )

            def sv_transp(ch):
                nc.scalar.dma_start_transpose(
                    svsT8[ch].bitcast(u16),
                    s8d.bitcast(u16)[ch * RCH:(ch + 1) * RCH, :],
                )

            def sv_square(ch):
                # split the 4 512-n slices between Pool and DVE
                for q in range(RCH // NCH):
                    lo = 2 * (q * NCH)
                    hi = lo + 2 * NCH
                    eng = nc.gpsimd if q % 2 == 0 else nc.vector
                    eng.tensor_tensor(
                        sqT8[ch][:, :, lo:hi],
                        svsT8[ch][:, :, lo:hi],
                        svsT8[ch][:, :, lo:hi],
                        ALU.mult,
                    )

            # ---------- emission ----------
            x_load(0)
            x_sq_act(0)
            sv_load(0)
            x_load(1)
            sv_load(1)
            x_cast(0)
            x_store_transp(0)
            sv_cast_store(0)
            sv_transp(0)
            sv_square(0)
            x2sc_piece(0, nc.vector)
            x_cast(1)
            x_store_transp(1)
            sv_cast_store(1)
            sv_transp(1)
            sv_square(1)
            x_sq_dve(1)
            x2sc_piece(1, nc.vector)

            # ---------- main loop ----------
            def mm_group(g):
                for t in range(MT):
                    pw = pp.tile([P, NG], f32, tag="pw")
                    for c in range(NG // NCH):
                        n0 = c * NCH
                        bank = pw[:, c * NCH:(c + 1) * NCH]
                        for K in range(2):
                            nc.tensor.matmul(
                                bank,
                                xT8p[:, K, :, t * P:(t + 1) * P],
                                drpair(svsT8[g][:, K, :], n0, NCH),
                                start=(K == 0),
                                stop=False,
                                perf_mode=DR,
                            )
                        for K in range(2):
                            nc.tensor.matmul(
                                bank,
                                neg_q[:],
                                drpair(sqT8[g][:, K, :], n0, NCH),
                                start=False,
                                stop=(K == 1),
                                perf_mode=DR,
                            )
                    nc.scalar.activation(
                        pw[:], pw[:], AF.Exp,
                        bias=x2sc[:, t:t + 1], scale=scale_bc[:],
                        accum_out=partials[:, t, g:g + 1],
                    )

            mm_group(0)
            mm_group(1)

            # ---------- finalize: T = sum of group partials ----------
            out_all = cp.tile([P, MT], f32)
            for t in range(MT):
                nc.vector.tensor_tensor(
                    out_all[:, t:t + 1],
                    partials[:, t, 0:1], partials[:, t, 1:2], ALU.add,
                )
            nc.sync.dma_start(outd.rearrange("(t p) -> p t", p=P), out_all[:])

    nc.finalize()
    return nc


def kernel(X: np.ndarray, svs: np.ndarray, scale: np.ndarray) -> np.ndarray:
    from concourse.bass_utils import run_bass_kernel_spmd

    if "nc" not in _CACHE:
        _CACHE["nc"] = _build_nc()
    nc = _CACHE["nc"]

    X = np.ascontiguousarray(X, dtype=np.float32)
    svs = np.ascontiguousarray(svs, dtype=np.float32)
    sc = np.asarray(scale, dtype=np.float32).reshape(1)

    in_maps = [
        {
            "X": X[(i % XSH) * M_LOC:(i % XSH + 1) * M_LOC],
            "svs": svs[(i // XSH) * N_LOC:(i // XSH + 1) * N_LOC],
            "scale": sc,
        }
        for i in range(NCORES)
    ]
    res = run_bass_kernel_spmd(nc, in_maps, core_ids=list(range(NCORES)))
    T = [r["out"].reshape(M_LOC).astype(np.float64) for r in res.results]
    C = float(-np.log(N_FULL) + (D / 2) * np.log(float(sc[0]) / np.pi))
    out = np.concatenate(
        [np.log(T[q] + T[q + XSH]) + C for q in range(XSH)]
    )
    return out.astype(np.float32)


# revision 40
# speedup vs baseline: 2.4991x; 1.1708x over previous
"""KDE log-density kernel for Trainium2, SPMD across 8 NeuronCores.

Computes log_p[m] = logsumexp_n(-scale * ||X[m] - svs[n]||^2) - log(N)
                    + (D/2) * log(scale/pi)

Sharding: 4-way over X rows x 2-way over svs rows (core i handles X
quarter i%4 against svs half i//4).  Each core returns the raw partial
sum T[m] = sum_n exp(-scale*||x_m - s_n||^2) over its svs half; the
host unshards by summing the two halves per query row and applying
log(T) + C.  This halves per-core DMA versus replicating svs, which is
what the kernel is otherwise bound by.

Per-core algorithm (fp8 DoubleRow pipeline):
  - svs (2 chunks of 2048 rows = one n-group each): DMA f32 load ->
    DVE cast 2*svs to fp8e4 -> DMA store fp8 -> DMA-transpose the
    uint16-bitcast (adjacent-d pairs packed per 16-bit element) into
    per-group svsT8 [128, 2, 2*NG] fp8 where (partition p, K, lane i)
    maps to d = 2*(K*128+p)+i.  Squares of svsT8 (for the -s2 inject)
    alternate Pool/DVE per 512-n slice.  Identical packing on both
    matmul operands keeps the DoubleRow contraction consistent.
  - X: f32 load in halves; x2 row-sums for half 0 on the idle ACT head
    (Square+accum), half 1 on DVE; cast/store/transpose per half.
  - Main loop over 2 n-groups x 16 m-tiles: per 512-n chunk, 2
    DoubleRow matmuls (K=256 each) accumulate 2*x.s into a psum bank,
    then 2 DoubleRow matmuls with an all-(-0.25) stationary against
    sqT8 add -s2[n] into the same bank.  One wide ACT exp over 4 banks
    [128, 2048] in-place with bias=-scale*x2[m], scale=scale,
    accum_out -> partials.  T = sum of the 2 group partials.

DMA queues: loads with no dependencies on SP; dependent stores and
transposes issue from the ACT queue (issue-only on its sequencer).
"""

import sys

for _p in ("/opt/trn_rl_repo", "/opt/pypackages"):
    if _p not in sys.path:
        sys.path.insert(0, _p)

import numpy as np

M_FULL, N_FULL, D = 8192, 8192, 512
NCORES = 8
XSH, SSH = 4, 2           # X shards x svs shards
M_LOC = M_FULL // XSH     # 2048
N_LOC = N_FULL // SSH     # 4096
P = 128
MT = M_LOC // P           # 16 m-tiles per core
NCH = 512                 # psum bank of f32
NGRP = 2                  # n-groups (one wide psum tile each per m-tile)
NG = N_LOC // NGRP        # 2048 n per group
RCH = 2048                # svs rows per pipeline chunk (= one n-group)
JCH = RCH // P            # 16 row-tiles per chunk
XH = M_LOC // 2           # X half rows (1024)

_CACHE = {}


def _build_nc():
    import concourse.mybir as mybir
    import concourse.tile as tile
    from concourse import bacc

    f32 = mybir.dt.float32
    fp8 = mybir.dt.float8e4
    u16 = mybir.dt.uint16
    AF = mybir.ActivationFunctionType
    ALU = mybir.AluOpType

    DR = mybir.MatmulPerfMode.DoubleRow

    nc = bacc.Bacc(None, target_bir_lowering=False, debug=True)
    Xd = nc.declare_dram_parameter("X", [M_LOC, D], f32, isOutput=False)
    Sd = nc.declare_dram_parameter("svs", [N_LOC, D], f32, isOutput=False)
    scd = nc.declare_dram_parameter("scale", [1], f32, isOutput=False)
    outd = nc.declare_dram_parameter("out", [NGRP, M_LOC], f32, isOutput=True)

    def drpair(ap2d, n0, ncols):
        # fp8 [128, 2*cols] packed-pair slice -> DoubleRow [128, 2, ncols]
        return ap2d[:, 2 * n0:2 * (n0 + ncols)].rearrange(
            "p (n two) -> p two n", two=2
        )

    with tile.TileContext(nc) as tc:
        with (
            tc.tile_pool(name="const", bufs=1) as cp,
            tc.tile_pool(name="stage", bufs=2) as stp,
            tc.tile_pool(name="s8", bufs=2) as s8p,
            tc.tile_pool(name="small", bufs=4) as sp,
            tc.tile_pool(name="wpsum", bufs=2, space="PSUM") as pp,
            tc.tile_pool(name="dram", bufs=1, space="DRAM") as dp,
        ):
            # ---------- constants ----------
            scale_bc = cp.tile([P, 1], f32)
            nc.sync.dma_start(scale_bc[:], scd[None, :].to_broadcast((P, 1)))
            negscale = cp.tile([P, 1], f32)
            nc.scalar.mul(negscale[:], scale_bc[:], -1.0)
            neg_q = cp.tile([P, 2, P], fp8)
            nc.gpsimd.memset(neg_q[:], -0.25)

            # resident tensors
            xst = cp.tile([P, MT, D], f32)          # X f32, row t*128+p
            x8 = cp.tile([P, MT, D], fp8)
            xT8 = cp.tile([P, 2, M_LOC * 2], fp8)   # packed pairs
            xT8p = cp.tile([P, 2, 2, M_LOC], fp8)   # planar [p, K, lane, m]
            x2_all = cp.tile([P, MT], f32)
            x2sc = cp.tile([P, MT], f32)
            svsT8 = [cp.tile([P, 2, NG * 2], fp8, name=f"svsT8{g}")
                     for g in range(NGRP)]
            sqT8 = [cp.tile([P, 2, NG * 2], fp8, name=f"sqT8{g}")
                    for g in range(NGRP)]
            partials = cp.tile([P, MT, NGRP], f32)

            x8d = dp.tile([M_LOC, D], fp8)
            s8d = dp.tile([N_LOC, D], fp8)

            # ---------- pipeline stages ----------
            def x_load(h):
                nc.sync.dma_start(
                    xst[:, h * 8:(h + 1) * 8, :],
                    Xd[h * XH:(h + 1) * XH, :].rearrange(
                        "(t p) d -> p t d", p=P),
                )

            def x_cast(h):
                nc.vector.tensor_copy(
                    x8[:, h * 8:(h + 1) * 8, :], xst[:, h * 8:(h + 1) * 8, :]
                )

            def x_store_transp(h):
                nc.sync.dma_start(
                    x8d[h * XH:(h + 1) * XH, :].rearrange(
                        "(t p) d -> p t d", p=P),
                    x8[:, h * 8:(h + 1) * 8, :],
                )
                nc.sync.dma_start_transpose(
                    xT8.bitcast(u16)[:, :, h * XH:(h + 1) * XH],
                    x8d.bitcast(u16)[h * XH:(h + 1) * XH, :],
                )
                # de-interleave to planar for the dual-fp8 ldweights ISA
                for K in range(2):
                    for i in range(2):
                        nc.vector.tensor_copy(
                            xT8p[:, K, i, h * XH:(h + 1) * XH],
                            xT8[:, K, 2 * h * XH + i:2 * (h + 1) * XH:2],
                        )

            def x_sq_act(h):
                for t in range(h * 8, (h + 1) * 8):
                    xsq = sp.tile([P, D], f32, tag="xsq")
                    nc.scalar.activation(
                        xsq[:], xst[:, t, :], AF.Square,
                        accum_out=x2_all[:, t:t + 1],
                    )

            def x_sq_dve(h):
                for t in range(h * 8, (h + 1) * 8):
                    xsq = sp.tile([P, D], fp8, tag="xsq8")
                    nc.vector.scalar_tensor_tensor(
                        xsq[:], xst[:, t, :], 1.0, xst[:, t, :],
                        ALU.mult, ALU.mult, accum_out=x2_all[:, t:t + 1],
                    )

            def x2sc_piece(h, eng):
                eng.tensor_scalar(
                    x2sc[:, h * 8:(h + 1) * 8],
                    x2_all[:, h * 8:(h + 1) * 8],
                    negscale[:], 0.0, ALU.mult, ALU.add,
                )

            sv_stage = {}

            def sv_load(ch, half=None, eng=None):
                rows = RCH if half is None else RCH // 2
                r0 = ch * RCH + (0 if not half else RCH // 2)
                svst = stp.tile([P, rows // P, D], f32, tag=f"svst{rows}",
                                bufs=(2 if half is not None else 1))
                sv_stage[(ch, half)] = svst
                (eng or nc.sync).dma_start(
                    svst[:],
                    Sd[r0:r0 + rows, :].rearrange("(j p) d -> p j d", p=P),
                )

            def sv_cast_store(ch, half=None):
                svst = sv_stage.pop((ch, half))
                rows = RCH if half is None else RCH // 2
                r0 = ch * RCH + (0 if not half else RCH // 2)
                sv8 = s8p.tile([P, rows // P, D], fp8, tag=f"sv8{rows}",
                               bufs=(2 if half is not None else 1))
                nc.vector.tensor_scalar_mul(sv8[:], svst[:], 2.0)
                nc.scalar.dma_start(
                    s8d[r0:r0 + rows, :].rearrange("(j p) d -> p j d", p=P),
                    sv8[:],
                )

            def sv_cast_store2(ch):
                # halved cast/store/transpose off one staged load (SP queue)
                svst = sv_stage.pop((ch, None))
                H = JCH // 2
                for hh in range(2):
                    r0 = ch * RCH + hh * (RCH // 2)
                    sv8 = s8p.tile([P, H, D], fp8, tag="sv8h2")
                    nc.vector.tensor_scalar_mul(
                        sv8[:], svst[:, hh * H:(hh + 1) * H, :], 2.0)
                    nc.sync.dma_start(
                        s8d[r0:r0 + RCH // 2, :].rearrange(
                            "(j p) d -> p j d", p=P),
                        sv8[:],
                    )
                    nc.sync.dma_start_transpose(
                        svsT8[ch].bitcast(u16)[
                            :, :, hh * RCH // 2:(hh + 1) * RCH // 2],
                        s8d.bitcast(u16)[r0:r0 + RCH // 2, :],
                    )
                    for q in (0, 1):
                        lo = 2 * (hh * RCH // 2 + q * NCH)
                        hi = lo + 2 * NCH
                        eng = nc.gpsimd if q % 2 == 0 else nc.vector
                        eng.tensor_tensor(
                            sqT8[ch][:, :, lo:hi],
                            svsT8[ch][:, :, lo:hi],
                            svsT8[ch][:, :, lo:hi],
                            ALU.mult,
                        )

            def sv_transp(ch, half=None):
                rows = RCH if half is None else RCH // 2
                r0 = ch * RCH + (0 if not half else RCH // 2)
                c0 = r0 - ch * RCH
                nc.scalar.dma_start_transpose(
                    svsT8[ch].bitcast(u16)[:, :, c0:c0 + rows],
                    s8d.bitcast(u16)[r0:r0 + rows, :],
                )

            def sv_square(ch):
                # split the 4 512-n slices between Pool and DVE
                for q in range(RCH // NCH):
                    lo = 2 * (q * NCH)
                    hi = lo + 2 * NCH
                    eng = nc.gpsimd if q == 0 else nc.vector
                    eng.tensor_tensor(
                        sqT8[ch][:, :, lo:hi],
                        svsT8[ch][:, :, lo:hi],
                        svsT8[ch][:, :, lo:hi],
                        ALU.mult,
                    )

            # ---------- emission ----------
            x_load(0)
            x_sq_act(0)
            x_cast(0)
            sv_load(0, 0)
            sv_load(0, 1)
            sv_cast_store(0, 0)
            sv_transp(0, 0)
            sv_cast_store(0, 1)
            sv_transp(0, 1)
            sv_square(0)
            x_store_transp(0)
            x2sc_piece(0, nc.vector)
            with tc.tile_wait_until(0.024):
                x_load(1)
            with tc.tile_wait_until(0.029):
                sv_load(1)
            x_cast(1)
            x_store_transp(1)
            sv_cast_store(1)
            sv_transp(1)
            sv_square(1)
            x_sq_dve(1)
            x2sc_piece(1, nc.vector)

            # ---------- main loop ----------
            def mm_group(g):
                for t in range(MT):
                    pw = pp.tile([P, NG], f32, tag="pw")
                    for c in range(NG // NCH):
                        n0 = c * NCH
                        bank = pw[:, c * NCH:(c + 1) * NCH]
                        for K in range(2):
                            nc.tensor.matmul(
                                bank,
                                xT8p[:, K, :, t * P:(t + 1) * P],
                                drpair(svsT8[g][:, K, :], n0, NCH),
                                start=(K == 0),
                                stop=False,
                                perf_mode=DR,
                            )
                        for K in range(2):
                            nc.tensor.matmul(
                                bank,
                                neg_q[:],
                                drpair(sqT8[g][:, K, :], n0, NCH),
                                start=False,
                                stop=(K == 1),
                                perf_mode=DR,
                            )
                    nc.scalar.activation(
                        pw[:], pw[:], AF.Exp,
                        bias=x2sc[:, t:t + 1], scale=scale_bc[:],
                        accum_out=partials[:, t, g:g + 1],
                    )

            mm_group(0)
            nc.sync.dma_start(
                outd[0].rearrange("(t p) -> p t", p=P), partials[:, :, 0]
            )
            mm_group(1)
            nc.sync.dma_start(
                outd[1].rearrange("(t p) -> p t", p=P), partials[:, :, 1]
            )

    nc.finalize()
    return nc


def kernel(X: np.ndarray, svs: np.ndarray, scale: np.ndarray) -> np.ndarray:
    from concourse.bass_utils import run_bass_kernel_spmd

    if "nc" not in _CACHE:
        _CACHE["nc"] = _build_nc()
    nc = _CACHE["nc"]

    X = np.ascontiguousarray(X, dtype=np.float32)
    svs = np.ascontiguousarray(svs, dtype=np.float32)
    sc = np.asarray(scale, dtype=np.float32).reshape(1)

    in_maps = [
        {
            "X": X[(i % XSH) * M_LOC:(i % XSH + 1) * M_LOC],
            "svs": svs[(i // XSH) * N_LOC:(i // XSH + 1) * N_LOC],
            "scale": sc,
        }
        for i in range(NCORES)
    ]
    res = run_bass_kernel_spmd(nc, in_maps, core_ids=list(range(NCORES)))
    T = [r["out"].reshape(NGRP, M_LOC).astype(np.float64).sum(axis=0)
         for r in res.results]
    C = float(-np.log(N_FULL) + (D / 2) * np.log(float(sc[0]) / np.pi))
    out = np.concatenate(
        [np.log(T[q] + T[q + XSH]) + C for q in range(XSH)]
    )
    return out.astype(np.float32)
